# revision 31
# baseline (speedup 1.0000x reference)
"""Trainium2 kernel for nn_CompressedLinearRANS: out = x @ (w_int8*scale).T + bias.

v7 = v4 (int8 w staged + on-chip cast to fp16; fp8 DoubleRow tail) plus
trace-driven schedule fixes:
 - PE warm-up: a burst of garbage matmuls on a memset tile at kernel start
   flips the HAM clock gate (1.2->2.4 GHz) during the DMA preamble, so the
   real stream runs warm from its first matmul (v4 paid ~7us of cold
   matmuls until t=17us).
 - fp8 DoubleRow tail lengthened K2 1024->1280 (emulated rel err 0.0197 vs
   gate 2e-2; emulation matched HW to 5e-6 at K2=1024 and 1e-7 at 1280).
 - final evict split into 128-col pieces to pipeline the copy/add/store
   tail (saved ~4us in v5 measurement).
 - x8/w8 DMA enqueues deferred behind the startup-critical x16/ws0 pieces.

(An attempted v5/v6 with host-precast fp16 w regressed: doubling the w DMA
bytes saturated aggregate DMA bandwidth for the first ~40us and starved the
k-outer sweep. The int8+cast pipeline is the right structure.)

Layout/schedule as v4: x host-cast (fp16 + fp8 slices) DMAed straight into
SBUF tiles; w int8 cast on vector/scalar for the fp16 part, host-cast fp8
bytes DMAed for the DR part; scale on scalar, bias-add on vector,
stores on sync.
"""

import os
import sys
import types

import numpy as np
import ml_dtypes

import concourse.bass as bass
import concourse.mybir as mybir
import concourse.tile as tile
from concourse.bass_utils import run_bass_kernel_spmd

# ---------------------------------------------------------------------------
# Compat shim: some agent images lack the optional ``antenv.axon_hooks``
# module that ``run_bass_kernel_spmd(trace=True)`` imports under axon. If
# it's missing, install a minimal stand-in (and register the ctypes NTFF
# hook when the axon .so supports it) so tracing works instead of crashing.
# No-op when the real module exists.
# ---------------------------------------------------------------------------
try:  # pragma: no cover
    import antenv.axon_hooks  # noqa: F401
except ImportError:
    _HOOK = [None]
    _mod = types.ModuleType("antenv.axon_hooks")
    _mod.set_axon_ntff_profile_hook = lambda h: _HOOK.__setitem__(0, h)
    _mod.get_axon_ntff_profile_hook = lambda: _HOOK[0]
    sys.modules["antenv.axon_hooks"] = _mod
    try:
        import antenv

        antenv.axon_hooks = _mod
    except ImportError:
        pass
    try:
        from trn_agent_boot.trn_boot import _ntff_profile_via_ctypes

        _hook = _ntff_profile_via_ctypes("/opt/axon/libaxon_pjrt.so")
        if _hook is not None:
            _mod.set_axon_ntff_profile_hook(_hook)
    except Exception:
        pass

DR = mybir.MatmulPerfMode.DoubleRow

# ---------------------------------------------------------------------------


def _split_sync_waits(nc, max_waits=1):
    for fn in nc.m.functions:
        for bb in fn.blocks:
            out = []
            changed = False
            for inst in bb.instructions:
                si = inst.sync_info
                waits = list(si.on_wait) if si is not None and si.on_wait else []
                if len(waits) > max_waits:
                    changed = True
                    for w in waits[:-max_waits]:
                        nop = mybir.InstNoOp(
                            name=nc.get_next_instruction_name(),
                            sync_info=mybir.SyncInfo(on_wait=[w], on_update=[]),
                            bass_nofuse=True,
                            engine=inst.engine,
                        )
                        nc.register_instruction(nop)
                        out.append(nop)
                    inst.sync_info = mybir.SyncInfo(
                        on_wait=waits[-max_waits:],
                        on_update=list(si.on_update or []),
                    )
                out.append(inst)
            if changed:
                bb.instructions = out


# ---------------------------------------------------------------------------

P = 128
N_CORES = 8

OUT_F, IN_F = 4096, 4096
B, S = 4, 2048
M_TOTAL = B * S
M = M_TOTAL // N_CORES

F32 = mybir.dt.float32
F16 = mybir.dt.float16
F8 = mybir.dt.float8e4
I8 = mybir.dt.int8

NF = 512
MT = M // P               # 8
NT = OUT_F // NF          # 8

K2 = int(os.environ.get("KERNEL_K2", "1280"))   # fp8 tail length
K1 = IN_F - K2
KO1 = K1 // P             # fp16 k-chunks
KO2 = K2 // P             # fp8 k-chunks (DR consumes pairs)
assert KO2 % 2 == 0

HEAD = 6                  # first nt=0 k-chunks host-precast to fp16

N_WARMUP = int(os.environ.get("KERNEL_WARMUP", "10"))

LAST_RESULTS = None


def _pieces(total, first=(2, 2), rest=4):
    out = list(first)
    while sum(out) < total:
        out.append(min(rest, total - sum(out)))
    return out


def build_nc():
    nc = bass.Bass()
    x16d = nc.dram_tensor("x16", [P, KO1, M], F16, kind="ExternalInput")
    x8d = nc.dram_tensor("x8", [P, KO2, M], F8, kind="ExternalInput")
    w16h = nc.dram_tensor("w16h", [P, HEAD, NF], F16, kind="ExternalInput")
    wp = nc.dram_tensor("wp", [P, NT, KO1, NF], I8, kind="ExternalInput")
    w8p = nc.dram_tensor("w8p", [P, NT, KO2, NF], F8, kind="ExternalInput")
    bias_rep = nc.dram_tensor("bias_rep", [P, OUT_F], F32, kind="ExternalInput")
    scale_rep = nc.dram_tensor("scale_rep", [P, 1], F32, kind="ExternalInput")
    out = nc.dram_tensor("out", [M, OUT_F], F32, kind="ExternalOutput")

    out_view = out.rearrange("(mt p) n -> p mt n", p=P)

    with tile.TileContext(nc) as tc:
        with (
            tc.tile_pool(name="const", bufs=1) as const_pool,
            tc.tile_pool(name="x16", bufs=1) as x16_pool,
            tc.tile_pool(name="x8", bufs=1) as x8_pool,
            tc.tile_pool(name="w16", bufs=2) as w16_pool,
            tc.tile_pool(name="w8", bufs=2) as w8_pool,
            tc.tile_pool(name="wstage", bufs=2) as wstage_pool,
            tc.tile_pool(name="outsb", bufs=6) as out_pool,
            tc.tile_pool(name="psum", bufs=8, space="PSUM") as psum_pool,
        ):
            # ---- PE warm-up: flip the HAM clock gate during DMA preamble ----
            dummy = const_pool.tile([P, NF], F16)
            nc.gpsimd.memset(dummy[:], 0.0)
            ps_warm = psum_pool.tile([P, NF], F32, tag="ps", name="ps_warm")
            for _ in range(N_WARMUP):
                nc.tensor.matmul(
                    ps_warm[:], dummy[:, :P], dummy[:], start=True, stop=True
                )

            # ---- startup-critical DMAs: ws0 + x16, ko-matched interleave so
            # the DMA rings serve them in consumption order (the k-outer
            # sweep eats one ko chunk of BOTH every ~1.7us; a big x16 piece
            # enqueued ahead of a small ws0 piece starves the cast chain) ----
            w16s = {}
            w8s = {}
            wstages = {}

            x16 = x16_pool.tile([P, KO1, M], F16)
            wstages[0] = wstage_pool.tile([P, KO1, NF], I8, tag="ws", name="ws0")
            w16s[0] = w16_pool.tile([P, KO1, NF], F16, tag="w16", name="w16_0")
            # head chunks arrive as ready-to-use fp16 (no cast dependency at
            # the stream head - the cast chain's first DMA round-trips are
            # too slow to feed the sweep from t~11us). Each hw DMA queue
            # (sync/scalar/gpsimd only) processes its DMAs serially at
            # ~2.5-3us latency apiece, so enqueue strictly in need order,
            # round-robin across all three queues.
            # 1-chunk pieces for the first 6 ko: per-queue DMA cadence at
            # the head is latency-bound (~2-3us per dma_start regardless of
            # size), so small pieces deliver one ko per cadence tick per
            # queue instead of stalling the sweep on a big piece's receipt
            for h in range(HEAD):
                nc.sync.dma_start(w16s[0][:, h : h + 1, :], w16h[:, h : h + 1, :])
                eng = nc.gpsimd if h % 2 == 0 else nc.scalar
                eng.dma_start(x16[:, h : h + 1, :], x16d[:, h : h + 1, :])
            lo = HEAD
            for i, kg in enumerate(_pieces(KO1 - HEAD, first=(4,), rest=4)):
                nc.sync.dma_start(
                    wstages[0][:, lo : lo + kg, :], wp[:, 0, lo : lo + kg, :]
                )
                eng = nc.gpsimd if i % 2 == 0 else nc.scalar
                eng.dma_start(x16[:, lo : lo + kg, :], x16d[:, lo : lo + kg, :])
                lo += kg

            def emit_w_dma(nt, trig=None):
                assert nt >= 1
                ws = wstage_pool.tile([P, KO1, NF], I8, tag="ws", name=f"ws{nt}")
                wstages[nt] = ws
                if trig is not None:
                    nc.gpsimd.tensor_copy(ws[:, 0, 0:1], trig)
                nc.sync.dma_start(ws[:], wp[:, nt, :, :])
                w8t = w8_pool.tile([P, KO2, NF], F8, tag="w8", name=f"w8_{nt}")
                w8s[nt] = w8t
                nc.sync.dma_start(w8t[:], w8p[:, nt, :, :])

            def emit_w_casts(nt, ko_lo, ko_hi):
                w16 = w16s[nt]
                ws = wstages[nt]
                for ko in range(ko_lo, ko_hi):
                    if ko % 2 == 0:
                        nc.scalar.copy(w16[:, ko, :], ws[:, ko, :])
                    else:
                        nc.vector.tensor_copy(w16[:, ko, :], ws[:, ko, :])

            w16s[1] = w16_pool.tile([P, KO1, NF], F16, tag="w16", name="w16_1")
            for ko in range(HEAD, KO1):
                nc.vector.tensor_copy(w16s[0][:, ko, :], wstages[0][:, ko, :])

            # ---- deferred bulk DMAs ----
            # x8/w8_0 aren't consumed until the DR section (~50us in), ws1/
            # w8_1/bias later still. Enqueue order alone can't stop their
            # transfers from jumping ahead of the startup-critical ws0/x16
            # pieces in the DMA rings (measured: a 6.5us PE gap at t~22us).
            # Gate each on the last x16 piece via a 1-element dummy write to
            # the target tile - the WAW dependency holds the transfer until
            # the critical stream has drained, still far ahead of first use.
            trig = x16[:, KO1 - 1, 0:1]
            w8s[0] = w8_pool.tile([P, KO2, NF], F8, tag="w8", name="w8_0")
            nc.scalar.copy(w8s[0][:, 0, 0:1], trig)
            nc.scalar.dma_start(w8s[0][:], w8p[:, 0, :, :])
            x8 = x8_pool.tile([P, KO2, M], F8)
            nc.scalar.copy(x8[:, 0, 0:1], trig)
            nc.scalar.dma_start(x8[:], x8d[:])
            emit_w_dma(1, trig=trig)
            bias_sb = const_pool.tile([P, OUT_F], F32)
            nc.sync.dma_start(bias_sb[:], bias_rep[:])
            scale_sb = const_pool.tile([P, 1], F32)
            nc.sync.dma_start(scale_sb[:], scale_rep[:])

            def evict(ps, mt, nt, split=1):
                nw = NF // split
                for s in range(split):
                    ot = out_pool.tile([P, nw], F32)
                    nc.scalar.activation(
                        ot[:], ps[:, s * nw : (s + 1) * nw],
                        mybir.ActivationFunctionType.Copy,
                        scale=scale_sb[:],
                    )
                    nc.vector.tensor_add(
                        ot[:], ot[:],
                        bias_sb[:, nt * NF + s * nw : nt * NF + (s + 1) * nw],
                    )
                    nc.sync.dma_start(
                        out_view[:, mt, nt * NF + s * nw : nt * NF + (s + 1) * nw],
                        ot[:],
                    )

            def dr_mms(ps, mt, nt):
                for j in range(KO2 // 2):
                    nc.tensor.matmul(
                        ps[:],
                        x8[:, 2 * j : 2 * j + 2, mt * P : (mt + 1) * P],
                        w8s[nt][:, 2 * j : 2 * j + 2, :],
                        start=False,
                        stop=(j == KO2 // 2 - 1),
                        perf_mode=DR,
                    )

            # ---- nt=0: k-outer across all 8 psum banks ----
            pss = [
                psum_pool.tile([P, NF], F32, tag="ps", name=f"ps0_{mt}")
                for mt in range(MT)
            ]
            for ko in range(KO1):
                if ko < KO1 // 2:
                    emit_w_casts(1, 2 * ko, min(2 * ko + 2, KO1))
                for mt in range(MT):
                    nc.tensor.matmul(
                        pss[mt][:],
                        x16[:, ko, mt * P : (mt + 1) * P],
                        w16s[0][:, ko, :],
                        start=(ko == 0),
                        stop=False,
                    )
            for j in range(KO2 // 2):
                for mt in range(MT):
                    nc.tensor.matmul(
                        pss[mt][:],
                        x8[:, 2 * j : 2 * j + 2, mt * P : (mt + 1) * P],
                        w8s[0][:, 2 * j : 2 * j + 2, :],
                        start=False,
                        stop=(j == KO2 // 2 - 1),
                        perf_mode=DR,
                    )

            for mt in range(MT):
                evict(pss[mt], mt, 0)

            # ---- nt >= 1: mt-outer, k-inner ----
            for nt in range(1, NT):
                if nt + 1 < NT:
                    emit_w_dma(nt + 1)
                    w16s[nt + 1] = w16_pool.tile(
                        [P, KO1, NF], F16, tag="w16", name=f"w16_{nt + 1}"
                    )
                for mt in range(MT):
                    last = nt == NT - 1 and mt == MT - 1
                    if not last:
                        ps = psum_pool.tile(
                            [P, NF], F32, tag="ps", name=f"ps{nt}_{mt}"
                        )
                        for ko in range(KO1):
                            nc.tensor.matmul(
                                ps[:],
                                x16[:, ko, mt * P : (mt + 1) * P],
                                w16s[nt][:, ko, :],
                                start=(ko == 0),
                                stop=False,
                            )
                        dr_mms(ps, mt, nt)
                        if nt + 1 < NT:
                            ncast = (KO1 + MT - 1) // MT
                            emit_w_casts(
                                nt + 1, mt * ncast, min(mt * ncast + ncast, KO1)
                            )
                        evict(ps, mt, nt)
                        continue
                    # final group: two N=256 halves so the first half's
                    # evict chain overlaps the second half's matmuls and
                    # only a half-width evict trails the last matmul
                    NH = NF // 2
                    for h in range(2):
                        psh = psum_pool.tile(
                            [P, NH], F32, tag="ps", name=f"ps{nt}_{mt}_h{h}"
                        )
                        hs = h * NH
                        for ko in range(KO1):
                            nc.tensor.matmul(
                                psh[:],
                                x16[:, ko, mt * P : (mt + 1) * P],
                                w16s[nt][:, ko, hs : hs + NH],
                                start=(ko == 0),
                                stop=False,
                            )
                        for j in range(KO2 // 2):
                            nc.tensor.matmul(
                                psh[:],
                                x8[:, 2 * j : 2 * j + 2, mt * P : (mt + 1) * P],
                                w8s[nt][:, 2 * j : 2 * j + 2, hs : hs + NH],
                                start=False,
                                stop=(j == KO2 // 2 - 1),
                                perf_mode=DR,
                            )
                        for s in range(2):
                            nw = NH // 2
                            ot = out_pool.tile([P, nw], F32)
                            off = nt * NF + hs + s * nw
                            nc.scalar.activation(
                                ot[:], psh[:, s * nw : (s + 1) * nw],
                                mybir.ActivationFunctionType.Copy,
                                scale=scale_sb[:],
                            )
                            nc.vector.tensor_add(
                                ot[:], ot[:], bias_sb[:, off : off + nw]
                            )
                            (nc.sync if s == 0 else nc.gpsimd).dma_start(
                                out_view[:, mt, off : off + nw], ot[:]
                            )

    _split_sync_waits(nc)
    return nc


def kernel(x, weight_int8, scale, bias):
    global LAST_RESULTS
    x = np.asarray(x)
    weight_int8 = np.asarray(weight_int8)
    scale = np.asarray(scale)
    bias = np.asarray(bias)

    x2d = x.reshape(M_TOTAL, IN_F)
    x16 = x2d[:, :K1].astype(np.float16)                     # [M_TOTAL, K1]
    x8 = x2d[:, K1:].astype(np.float32).astype(ml_dtypes.float8_e4m3)

    w = weight_int8                                          # [N, K] int32
    w16part = w[:, :K1].astype(np.int8)
    wp = np.ascontiguousarray(
        w16part.reshape(NT, NF, KO1, P).transpose(3, 0, 2, 1)
    )
    # fp16 head of (nt=0, ko<HEAD): [P, HEAD, NF]
    w16h = np.ascontiguousarray(
        w[:NF, : HEAD * P].astype(np.float16).reshape(NF, HEAD, P).transpose(2, 1, 0)
    )
    w8part = w[:, K1:].astype(np.float32).astype(ml_dtypes.float8_e4m3)
    w8p = np.ascontiguousarray(
        w8part.reshape(NT, NF, KO2, P).transpose(3, 0, 2, 1)
    )

    bias_rep = np.ascontiguousarray(
        np.broadcast_to(bias.astype(np.float32, copy=False), (P, OUT_F))
    )
    scale_rep = np.full((P, 1), np.float32(scale), dtype=np.float32)

    in_maps = []
    for c in range(N_CORES):
        sh16 = x16[c * M : (c + 1) * M]
        sh8 = x8[c * M : (c + 1) * M]
        in_maps.append(
            {
                "x16": np.ascontiguousarray(
                    sh16.reshape(M, KO1, P).transpose(2, 1, 0)
                ),
                "x8": np.ascontiguousarray(
                    sh8.reshape(M, KO2, P).transpose(2, 1, 0)
                ),
                "w16h": w16h,
                "wp": wp,
                "w8p": w8p,
                "bias_rep": bias_rep,
                "scale_rep": scale_rep,
            }
        )

    nc = build_nc()
    trace = bool(int(os.environ.get("KERNEL_TRACE", "0")))
    res = run_bass_kernel_spmd(nc, in_maps, list(range(N_CORES)), trace=trace)
    LAST_RESULTS = res

    out = np.empty((M_TOTAL, OUT_F), dtype=np.float32)
    for c in range(N_CORES):
        out[c * M : (c + 1) * M] = res.results[c]["out"]
    return out.reshape(B, S, OUT_F)


# revision 32
# speedup vs baseline: 1.0192x; 1.0192x over previous
"""Trainium2 kernel for nn_CompressedLinearRANS: out = x @ (w_int8*scale).T + bias.

v7 = v4 (int8 w staged + on-chip cast to fp16; fp8 DoubleRow tail) plus
trace-driven schedule fixes:
 - PE warm-up: a burst of garbage matmuls on a memset tile at kernel start
   flips the HAM clock gate (1.2->2.4 GHz) during the DMA preamble, so the
   real stream runs warm from its first matmul (v4 paid ~7us of cold
   matmuls until t=17us).
 - fp8 DoubleRow tail lengthened K2 1024->1280 (emulated rel err 0.0197 vs
   gate 2e-2; emulation matched HW to 5e-6 at K2=1024 and 1e-7 at 1280).
 - final evict split into 128-col pieces to pipeline the copy/add/store
   tail (saved ~4us in v5 measurement).
 - x8/w8 DMA enqueues deferred behind the startup-critical x16/ws0 pieces.

(An attempted v5/v6 with host-precast fp16 w regressed: doubling the w DMA
bytes saturated aggregate DMA bandwidth for the first ~40us and starved the
k-outer sweep. The int8+cast pipeline is the right structure.)

Layout/schedule as v4: x host-cast (fp16 + fp8 slices) DMAed straight into
SBUF tiles; w int8 cast on vector/scalar for the fp16 part, host-cast fp8
bytes DMAed for the DR part; scale on scalar, bias-add on vector,
stores on sync.
"""

import os
import sys
import types

import numpy as np
import ml_dtypes

import concourse.bass as bass
import concourse.mybir as mybir
import concourse.tile as tile
from concourse.bass_utils import run_bass_kernel_spmd

# ---------------------------------------------------------------------------
# Compat shim: some agent images lack the optional ``antenv.axon_hooks``
# module that ``run_bass_kernel_spmd(trace=True)`` imports under axon. If
# it's missing, install a minimal stand-in (and register the ctypes NTFF
# hook when the axon .so supports it) so tracing works instead of crashing.
# No-op when the real module exists.
# ---------------------------------------------------------------------------
try:  # pragma: no cover
    import antenv.axon_hooks  # noqa: F401
except ImportError:
    _HOOK = [None]
    _mod = types.ModuleType("antenv.axon_hooks")
    _mod.set_axon_ntff_profile_hook = lambda h: _HOOK.__setitem__(0, h)
    _mod.get_axon_ntff_profile_hook = lambda: _HOOK[0]
    sys.modules["antenv.axon_hooks"] = _mod
    try:
        import antenv

        antenv.axon_hooks = _mod
    except ImportError:
        pass
    try:
        from trn_agent_boot.trn_boot import _ntff_profile_via_ctypes

        _hook = _ntff_profile_via_ctypes("/opt/axon/libaxon_pjrt.so")
        if _hook is not None:
            _mod.set_axon_ntff_profile_hook(_hook)
    except Exception:
        pass

DR = mybir.MatmulPerfMode.DoubleRow

# ---------------------------------------------------------------------------


def _split_sync_waits(nc, max_waits=1):
    for fn in nc.m.functions:
        for bb in fn.blocks:
            out = []
            changed = False
            for inst in bb.instructions:
                si = inst.sync_info
                waits = list(si.on_wait) if si is not None and si.on_wait else []
                if len(waits) > max_waits:
                    changed = True
                    for w in waits[:-max_waits]:
                        nop = mybir.InstNoOp(
                            name=nc.get_next_instruction_name(),
                            sync_info=mybir.SyncInfo(on_wait=[w], on_update=[]),
                            bass_nofuse=True,
                            engine=inst.engine,
                        )
                        nc.register_instruction(nop)
                        out.append(nop)
                    inst.sync_info = mybir.SyncInfo(
                        on_wait=waits[-max_waits:],
                        on_update=list(si.on_update or []),
                    )
                out.append(inst)
            if changed:
                bb.instructions = out


# ---------------------------------------------------------------------------

P = 128
N_CORES = 8

OUT_F, IN_F = 4096, 4096
B, S = 4, 2048
M_TOTAL = B * S
M = M_TOTAL // N_CORES

F32 = mybir.dt.float32
F16 = mybir.dt.float16
F8 = mybir.dt.float8e4
I8 = mybir.dt.int8

NF = 512
MT = M // P               # 8
NT = OUT_F // NF          # 8

K2 = int(os.environ.get("KERNEL_K2", "1280"))   # fp8 tail length
K1 = IN_F - K2
KO1 = K1 // P             # fp16 k-chunks
KO2 = K2 // P             # fp8 k-chunks (DR consumes pairs)
assert KO2 % 2 == 0

HEAD = 6                  # first nt=0 k-chunks host-precast to fp16

N_WARMUP = int(os.environ.get("KERNEL_WARMUP", "10"))

LAST_RESULTS = None


def _pieces(total, first=(2, 2), rest=4):
    out = list(first)
    while sum(out) < total:
        out.append(min(rest, total - sum(out)))
    return out


def build_nc():
    nc = bass.Bass()
    x16d = nc.dram_tensor("x16", [P, KO1, M], F16, kind="ExternalInput")
    x8d = nc.dram_tensor("x8", [P, KO2, M], F8, kind="ExternalInput")
    w16h = nc.dram_tensor("w16h", [P, HEAD, NF], F16, kind="ExternalInput")
    wp = nc.dram_tensor("wp", [P, NT, KO1, NF], I8, kind="ExternalInput")
    w8p = nc.dram_tensor("w8p", [P, NT, KO2, NF], F8, kind="ExternalInput")
    bias_rep = nc.dram_tensor("bias_rep", [P, OUT_F], F32, kind="ExternalInput")
    scale_rep = nc.dram_tensor("scale_rep", [P, 1], F32, kind="ExternalInput")
    out = nc.dram_tensor("out", [M, OUT_F], F32, kind="ExternalOutput")

    out_view = out.rearrange("(mt p) n -> p mt n", p=P)

    with tile.TileContext(nc) as tc:
        with (
            tc.tile_pool(name="const", bufs=1) as const_pool,
            tc.tile_pool(name="x16", bufs=1) as x16_pool,
            tc.tile_pool(name="x8", bufs=1) as x8_pool,
            tc.tile_pool(name="w16", bufs=2) as w16_pool,
            tc.tile_pool(name="w8", bufs=2) as w8_pool,
            tc.tile_pool(name="wstage", bufs=2) as wstage_pool,
            tc.tile_pool(name="outsb", bufs=6) as out_pool,
            tc.tile_pool(name="psum", bufs=8, space="PSUM") as psum_pool,
        ):
            # ---- PE warm-up: flip the HAM clock gate during DMA preamble ----
            dummy = const_pool.tile([P, NF], F16)
            nc.gpsimd.memset(dummy[:], 0.0)
            ps_warm = psum_pool.tile([P, NF], F32, tag="ps", name="ps_warm")
            for _ in range(N_WARMUP):
                nc.tensor.matmul(
                    ps_warm[:], dummy[:, :P], dummy[:], start=True, stop=True
                )

            # ---- startup-critical DMAs: ws0 + x16, ko-matched interleave so
            # the DMA rings serve them in consumption order (the k-outer
            # sweep eats one ko chunk of BOTH every ~1.7us; a big x16 piece
            # enqueued ahead of a small ws0 piece starves the cast chain) ----
            w16s = {}
            w8s = {}
            wstages = {}

            x16 = x16_pool.tile([P, KO1, M], F16)
            wstages[0] = wstage_pool.tile([P, KO1, NF], I8, tag="ws", name="ws0")
            w16s[0] = w16_pool.tile([P, KO1, NF], F16, tag="w16", name="w16_0")
            # head chunks arrive as ready-to-use fp16 (no cast dependency at
            # the stream head - the cast chain's first DMA round-trips are
            # too slow to feed the sweep from t~11us). Each hw DMA queue
            # (sync/scalar/gpsimd only) processes its DMAs serially at
            # ~2.5-3us latency apiece, so enqueue strictly in need order,
            # round-robin across all three queues.
            nc.sync.dma_start(w16s[0][:, 0:2, :], w16h[:, 0:2, :])
            nc.gpsimd.dma_start(x16[:, 0:1, :], x16d[:, 0:1, :])
            nc.scalar.dma_start(x16[:, 1:2, :], x16d[:, 1:2, :])
            nc.sync.dma_start(w16s[0][:, 2:4, :], w16h[:, 2:4, :])
            nc.gpsimd.dma_start(x16[:, 2:4, :], x16d[:, 2:4, :])
            nc.scalar.dma_start(x16[:, 4:6, :], x16d[:, 4:6, :])
            nc.sync.dma_start(w16s[0][:, 4:6, :], w16h[:, 4:6, :])
            lo = HEAD
            for i, kg in enumerate(_pieces(KO1 - HEAD, first=(4,), rest=4)):
                nc.sync.dma_start(
                    wstages[0][:, lo : lo + kg, :], wp[:, 0, lo : lo + kg, :]
                )
                eng = nc.gpsimd if i % 2 == 0 else nc.scalar
                eng.dma_start(x16[:, lo : lo + kg, :], x16d[:, lo : lo + kg, :])
                lo += kg

            def emit_w_dma(nt, trig=None):
                assert nt >= 1
                ws = wstage_pool.tile([P, KO1, NF], I8, tag="ws", name=f"ws{nt}")
                wstages[nt] = ws
                if trig is not None:
                    nc.gpsimd.tensor_copy(ws[:, 0, 0:1], trig)
                nc.sync.dma_start(ws[:], wp[:, nt, :, :])
                w8t = w8_pool.tile([P, KO2, NF], F8, tag="w8", name=f"w8_{nt}")
                w8s[nt] = w8t
                nc.sync.dma_start(w8t[:], w8p[:, nt, :, :])

            def emit_w_casts(nt, ko_lo, ko_hi):
                w16 = w16s[nt]
                ws = wstages[nt]
                for ko in range(ko_lo, ko_hi):
                    if ko % 2 == 0:
                        nc.scalar.copy(w16[:, ko, :], ws[:, ko, :])
                    else:
                        nc.vector.tensor_copy(w16[:, ko, :], ws[:, ko, :])

            w16s[1] = w16_pool.tile([P, KO1, NF], F16, tag="w16", name="w16_1")
            for ko in range(HEAD, KO1):
                nc.vector.tensor_copy(w16s[0][:, ko, :], wstages[0][:, ko, :])

            # ---- deferred bulk DMAs ----
            # x8/w8_0 aren't consumed until the DR section (~50us in), ws1/
            # w8_1/bias later still. Enqueue order alone can't stop their
            # transfers from jumping ahead of the startup-critical ws0/x16
            # pieces in the DMA rings (measured: a 6.5us PE gap at t~22us).
            # Gate each on the last x16 piece via a 1-element dummy write to
            # the target tile - the WAW dependency holds the transfer until
            # the critical stream has drained, still far ahead of first use.
            trig = x16[:, KO1 - 1, 0:1]
            w8s[0] = w8_pool.tile([P, KO2, NF], F8, tag="w8", name="w8_0")
            nc.scalar.copy(w8s[0][:, 0, 0:1], trig)
            nc.scalar.dma_start(w8s[0][:], w8p[:, 0, :, :])
            x8 = x8_pool.tile([P, KO2, M], F8)
            nc.scalar.copy(x8[:, 0, 0:1], trig)
            nc.scalar.dma_start(x8[:], x8d[:])
            emit_w_dma(1, trig=trig)
            bias_sb = const_pool.tile([P, OUT_F], F32)
            nc.sync.dma_start(bias_sb[:], bias_rep[:])
            scale_sb = const_pool.tile([P, 1], F32)
            nc.sync.dma_start(scale_sb[:], scale_rep[:])

            def evict(ps, mt, nt, split=1):
                nw = NF // split
                for s in range(split):
                    ot = out_pool.tile([P, nw], F32)
                    nc.scalar.activation(
                        ot[:], ps[:, s * nw : (s + 1) * nw],
                        mybir.ActivationFunctionType.Copy,
                        scale=scale_sb[:],
                    )
                    nc.vector.tensor_add(
                        ot[:], ot[:],
                        bias_sb[:, nt * NF + s * nw : nt * NF + (s + 1) * nw],
                    )
                    nc.sync.dma_start(
                        out_view[:, mt, nt * NF + s * nw : nt * NF + (s + 1) * nw],
                        ot[:],
                    )

            def dr_mms(ps, mt, nt):
                for j in range(KO2 // 2):
                    nc.tensor.matmul(
                        ps[:],
                        x8[:, 2 * j : 2 * j + 2, mt * P : (mt + 1) * P],
                        w8s[nt][:, 2 * j : 2 * j + 2, :],
                        start=False,
                        stop=(j == KO2 // 2 - 1),
                        perf_mode=DR,
                    )

            # ---- nt=0: k-outer across all 8 psum banks ----
            pss = [
                psum_pool.tile([P, NF], F32, tag="ps", name=f"ps0_{mt}")
                for mt in range(MT)
            ]
            for ko in range(KO1):
                if ko < KO1 // 2:
                    emit_w_casts(1, 2 * ko, min(2 * ko + 2, KO1))
                for mt in range(MT):
                    nc.tensor.matmul(
                        pss[mt][:],
                        x16[:, ko, mt * P : (mt + 1) * P],
                        w16s[0][:, ko, :],
                        start=(ko == 0),
                        stop=False,
                    )
            for j in range(KO2 // 2):
                for mt in range(MT):
                    nc.tensor.matmul(
                        pss[mt][:],
                        x8[:, 2 * j : 2 * j + 2, mt * P : (mt + 1) * P],
                        w8s[0][:, 2 * j : 2 * j + 2, :],
                        start=False,
                        stop=(j == KO2 // 2 - 1),
                        perf_mode=DR,
                    )

            for mt in range(MT):
                evict(pss[mt], mt, 0)

            # ---- nt >= 1: mt-outer, k-inner ----
            for nt in range(1, NT):
                if nt + 1 < NT:
                    emit_w_dma(nt + 1)
                    w16s[nt + 1] = w16_pool.tile(
                        [P, KO1, NF], F16, tag="w16", name=f"w16_{nt + 1}"
                    )
                for mt in range(MT):
                    last = nt == NT - 1 and mt == MT - 1
                    if not last:
                        ps = psum_pool.tile(
                            [P, NF], F32, tag="ps", name=f"ps{nt}_{mt}"
                        )
                        for ko in range(KO1):
                            nc.tensor.matmul(
                                ps[:],
                                x16[:, ko, mt * P : (mt + 1) * P],
                                w16s[nt][:, ko, :],
                                start=(ko == 0),
                                stop=False,
                            )
                        dr_mms(ps, mt, nt)
                        if nt + 1 < NT:
                            ncast = (KO1 + MT - 1) // MT
                            emit_w_casts(
                                nt + 1, mt * ncast, min(mt * ncast + ncast, KO1)
                            )
                        evict(ps, mt, nt)
                        continue
                    # final group: two N=256 halves so the first half's
                    # evict chain overlaps the second half's matmuls and
                    # only a half-width evict trails the last matmul
                    NH = NF // 2
                    for h in range(2):
                        psh = psum_pool.tile(
                            [P, NH], F32, tag="ps", name=f"ps{nt}_{mt}_h{h}"
                        )
                        hs = h * NH
                        for ko in range(KO1):
                            nc.tensor.matmul(
                                psh[:],
                                x16[:, ko, mt * P : (mt + 1) * P],
                                w16s[nt][:, ko, hs : hs + NH],
                                start=(ko == 0),
                                stop=False,
                            )
                        for j in range(KO2 // 2):
                            nc.tensor.matmul(
                                psh[:],
                                x8[:, 2 * j : 2 * j + 2, mt * P : (mt + 1) * P],
                                w8s[nt][:, 2 * j : 2 * j + 2, hs : hs + NH],
                                start=False,
                                stop=(j == KO2 // 2 - 1),
                                perf_mode=DR,
                            )
                        for s in range(2):
                            nw = NH // 2
                            ot = out_pool.tile([P, nw], F32)
                            off = nt * NF + hs + s * nw
                            nc.scalar.activation(
                                ot[:], psh[:, s * nw : (s + 1) * nw],
                                mybir.ActivationFunctionType.Copy,
                                scale=scale_sb[:],
                            )
                            nc.vector.tensor_add(
                                ot[:], ot[:], bias_sb[:, off : off + nw]
                            )
                            (nc.sync if s == 0 else nc.gpsimd).dma_start(
                                out_view[:, mt, off : off + nw], ot[:]
                            )

    _split_sync_waits(nc)
    return nc


def kernel(x, weight_int8, scale, bias):
    global LAST_RESULTS
    x = np.asarray(x)
    weight_int8 = np.asarray(weight_int8)
    scale = np.asarray(scale)
    bias = np.asarray(bias)

    x2d = x.reshape(M_TOTAL, IN_F)
    x16 = x2d[:, :K1].astype(np.float16)                     # [M_TOTAL, K1]
    x8 = x2d[:, K1:].astype(np.float32).astype(ml_dtypes.float8_e4m3)

    w = weight_int8                                          # [N, K] int32
    w16part = w[:, :K1].astype(np.int8)
    wp = np.ascontiguousarray(
        w16part.reshape(NT, NF, KO1, P).transpose(3, 0, 2, 1)
    )
    # fp16 head of (nt=0, ko<HEAD): [P, HEAD, NF]
    w16h = np.ascontiguousarray(
        w[:NF, : HEAD * P].astype(np.float16).reshape(NF, HEAD, P).transpose(2, 1, 0)
    )
    w8part = w[:, K1:].astype(np.float32).astype(ml_dtypes.float8_e4m3)
    w8p = np.ascontiguousarray(
        w8part.reshape(NT, NF, KO2, P).transpose(3, 0, 2, 1)
    )

    bias_rep = np.ascontiguousarray(
        np.broadcast_to(bias.astype(np.float32, copy=False), (P, OUT_F))
    )
    scale_rep = np.full((P, 1), np.float32(scale), dtype=np.float32)

    in_maps = []
    for c in range(N_CORES):
        sh16 = x16[c * M : (c + 1) * M]
        sh8 = x8[c * M : (c + 1) * M]
        in_maps.append(
            {
                "x16": np.ascontiguousarray(
                    sh16.reshape(M, KO1, P).transpose(2, 1, 0)
                ),
                "x8": np.ascontiguousarray(
                    sh8.reshape(M, KO2, P).transpose(2, 1, 0)
                ),
                "w16h": w16h,
                "wp": wp,
                "w8p": w8p,
                "bias_rep": bias_rep,
                "scale_rep": scale_rep,
            }
        )

    nc = build_nc()
    trace = bool(int(os.environ.get("KERNEL_TRACE", "0")))
    res = run_bass_kernel_spmd(nc, in_maps, list(range(N_CORES)), trace=trace)
    LAST_RESULTS = res

    out = np.empty((M_TOTAL, OUT_F), dtype=np.float32)
    for c in range(N_CORES):
        out[c * M : (c + 1) * M] = res.results[c]["out"]
    return out.reshape(B, S, OUT_F)
